# revision 28
# baseline (speedup 1.0000x reference)
"""Trainium2 Bass kernel for the Luong-attention layer (nn_AttentionLayer).

Math (reference):
    hs_proj = enc @ W_a.T + b_a                  # [S,B,H]
    scores[s,b] = hs_proj[s,b] . h_t[b]          # [S,B]
    scores += log(mask).T
    a = softmax(scores, axis=0)
    c_t[b] = sum_s a[s,b] * enc[s,b]             # [B,H]
    out = tanh([c_t, h_t] @ W_r.T + b_r)         # [B,H]

Restructuring:
  * scores[s,b] = enc[s,b] . u[b] with u = h_t @ W_a (b_a cancels in the
    s-axis softmax); softmax shift folded as a fixed constant C=40.
  * Data-parallel over batch: 8 cores x 8 batches, no collectives. Each
    core streams its enc shard ONCE from HBM as fp16 (32 MiB).
  * enc streamed in PAIRS of 128-row s-tiles (2 MiB per pair), the two
    1 MiB halves on the sync and scalar HW DMA queues so both queues
    saturate from the start; all weights/constants ride the gpsimd
    SWDGE queue packed into a handful of contiguous transfers.
  * Scores on DVE via the custom paged "SCAN_DOT" op (2 fp16
    elems/lane/cycle, compensated coarse/resid fp16 output pair per
    page) — ONE instruction per pair (16 pages), nothing else on DVE in
    steady state so the DVE (0.96 GHz) keeps pace with the 425 GB/s DMA
    stream.
  * The two small adds (page-sum extraction + diagonal-mask spread) run
    on the otherwise-idle Pool (gpsimd) engine; exp on ACT with
    accumulated denominators.
  * PE: psum += psp(bf16).T @ enc(fp16), alternating between two PE
    column-groups (tile_position) so LDWEIGHTS hides under the in-flight
    matmul; the final merge of the two accumulator row-sets is fused
    into the output transposes (stationary ct4 chunk x selector s2).
  * Epilogue: 1/l scaling deferred past the output projection and fused
    with the +oh add in one scalar_tensor_tensor; the last pair's
    exp/matmuls are split into quarters to shorten the post-DMA drain.
Per-core: partitions p = (s_sub 16, b 8), h on the free axis; enc
host-pre-permuted so each 1 MiB fp16 half-pair is one contiguous DMA.
"""

import sys

if "/opt/trn_rl_repo" not in sys.path:
    sys.path.insert(0, "/opt/trn_rl_repo")

import ml_dtypes
import numpy as np

import concourse.bacc as bacc
import concourse.dve_ops as dve_ops_mod
import concourse.mybir as mybir
from concourse import bass_isa, tile
from concourse.bass import assert_partition_dims_match
from concourse.bass_utils import run_bass_kernel_spmd
from concourse.dve_ops import DveOp
from concourse.dve_spec import C0, Spec, Src0, Src1, lower as dve_lower, scan
from concourse.dve_uop import (
    AluInp,
    AluOp as DveAluOp,
    DelayInp,
    DveOpSpec,
    InpSel,
    OutPath,
    OutSel,
    Trigger,
    UopConfig,
)

S, B, H = 4096, 64, 512
NCORES = 8
BC = B // NCORES          # 8 batches per core
SS = 128 // BC            # 16 s-positions per group
S_TILE = 128              # s-positions per tile
NPAIR = S // (2 * S_TILE)  # 16 DMA pairs (2 tiles each)
GPP = 2 * (S_TILE // SS)  # 16 groups per pair
PW = GPP * H              # pair width in elements (8192)
C_SHIFT = 40.0
NEG_INF = -1.0e30
F32 = mybir.dt.float32
F16 = mybir.dt.float16
BF16 = mybir.dt.bfloat16
I32 = mybir.dt.int32
AF = mybir.ActivationFunctionType
ALU = mybir.AluOpType

# --------------------------------------------------------------------------
# SCAN_DOT custom DVE op: fused fp16 mul + fp32 accumulate at 2 elems/cycle,
# emitting the running sum as a compensated (coarse, resid) fp16 pair.
# --------------------------------------------------------------------------

_PD = [AluInp.PREV_DELAY_0, AluInp.PREV_DELAY_1, AluInp.PREV_DELAY_2,
       AluInp.PREV_DELAY_3, AluInp.PREV_DELAY_4, AluInp.PREV_DELAY_5]

_SCAN_DOT_SPEC = Spec(
    body=scan(DveAluOp.ADD, Src0 * Src1, init=C0),
    reference=lambda in0, in1, s0, s1, imm2: (
        np.cumsum(in0.astype(np.float32) * in1.astype(np.float32), axis=-1) + s0
    ),
)


def _sd_inputs(u, two_x):
    u.enable_input(InpSel.SRC_0, 1)
    u.enable_input(InpSel.SRC_1, 2)
    if two_x:
        u.enable_input(InpSel.SRC_0_HI, 3)
        u.enable_input(InpSel.SRC_1_HI, 4)
    u.enable_input(InpSel.CONST_0, 5)
    u.enable_input(InpSel.MASK16_SL16, 6)


def _sd_state_2x(kind):
    """kind: 'seed' | 'steady' | 'step' (paged, 3-state FSM)."""
    u = UopConfig()
    _sd_inputs(u, two_x=True)
    dp = u.datapath_config
    if kind == "seed":
        dp[0].pass_through_alu().pass_through_delay(4)
        dp[1].pass_through_alu().pass_through_delay(4)
        dp[2].pass_through_alu().pass_through_delay(4)
        dp[3].enable_alu(DveAluOp.BYPASS, _PD[4], _PD[4])
        for i in range(4, 8):
            dp[i].pass_through_alu()
        u.repeat_count = 1
        u.trigger = (Trigger.COUNT, Trigger.NONE, Trigger.NONE)
        u.next_uop = (1, 0, 0)
        return u
    dp[0].enable_alu(DveAluOp.MULTIPLY, _PD[0], _PD[1]).pass_through_delay(2, 3, 5)
    dp[1].enable_alu(DveAluOp.MULTIPLY, _PD[2], _PD[3]).pass_through_delay(5)
    dp[1].enable_delay_from_src(DelayInp.PREV_ALU_OUT, 0)
    dp[2].enable_alu(DveAluOp.ADD, AluInp.PREV_ALU_OUT, _PD[0]).pass_through_delay(5)
    if kind == "steady":
        dp[3].enable_alu(DveAluOp.ADD, AluInp.CURR_ALU_OUT, AluInp.PREV_ALU_OUT)
    else:  # step: first pair of a new page -> acc = 0 + pair
        dp[3].enable_alu(DveAluOp.BYPASS, AluInp.PREV_ALU_OUT, AluInp.PREV_ALU_OUT)
    dp[3].pass_through_delay(5)
    dp[4].enable_alu(DveAluOp.BITWISE_AND, AluInp.PREV_ALU_OUT, _PD[5])
    dp[4].enable_delay_from_src(DelayInp.PREV_ALU_OUT, 0)
    dp[5].enable_alu(DveAluOp.SUBTRACT, _PD[0], AluInp.PREV_ALU_OUT)
    dp[5].enable_delay_from_src(DelayInp.PREV_ALU_OUT, 1)
    dp[6].pass_through_alu().pass_through_delay(1)
    dp[7].pass_through_alu().pass_through_delay(1)
    u.enable_output(OutSel.DELAY_1, OutPath.WR0_LO)   # coarse -> even col
    u.enable_output(OutSel.ALU_OUT, OutPath.WR0_HI)   # resid  -> odd col
    u.require_inp0 = 1
    u.require_inp1 = 1
    if kind == "steady":
        u.trigger = (Trigger.SRC_TENSOR_DONE, Trigger.SUB_DIM_DONE, Trigger.NONE)
        u.next_uop = (0, 2, 0)
    else:
        u.repeat_count = 1
        u.trigger = (Trigger.SRC_TENSOR_DONE, Trigger.SUB_DIM_DONE, Trigger.COUNT)
        u.next_uop = (0, 2, 1)
    return u


def _sd_state_1x(kind):
    """1x fallback twin (one elem/cycle, plain prefix per page; the last
    column of each page is the full page sum, col N-2 is prefix N-1 — the
    coarse+resid read degrades, so call sites must qualify for 2x; a 1x
    fallback is caught by the rel-err gate)."""
    u = UopConfig()
    _sd_inputs(u, two_x=False)
    dp = u.datapath_config
    if kind == "seed":
        dp[0].pass_through_alu().pass_through_delay(4)
        dp[1].pass_through_alu().pass_through_delay(4)
        dp[2].pass_through_alu().pass_through_delay(4)
        dp[3].enable_alu(DveAluOp.BYPASS, _PD[4], _PD[4])
        for i in range(4, 8):
            dp[i].pass_through_alu()
        u.repeat_count = 1
        u.trigger = (Trigger.COUNT, Trigger.NONE, Trigger.NONE)
        u.next_uop = (1, 0, 0)
        return u
    dp[0].enable_alu(DveAluOp.MULTIPLY, _PD[0], _PD[1])
    dp[1].pass_through_alu()
    dp[2].pass_through_alu()
    if kind == "steady":
        dp[3].enable_alu(DveAluOp.ADD, AluInp.CURR_ALU_OUT, AluInp.PREV_ALU_OUT)
    else:
        dp[3].enable_alu(DveAluOp.BYPASS, AluInp.PREV_ALU_OUT, AluInp.PREV_ALU_OUT)
    for i in range(4, 8):
        dp[i].pass_through_alu()
    u.enable_output(OutSel.ALU_OUT, OutPath.WR0_LO)
    u.require_inp0 = 1
    u.require_inp1 = 1
    if kind == "steady":
        u.trigger = (Trigger.SRC_TENSOR_DONE, Trigger.SUB_DIM_DONE, Trigger.NONE)
        u.next_uop = (0, 2, 0)
    else:
        u.repeat_count = 1
        u.trigger = (Trigger.SRC_TENSOR_DONE, Trigger.SUB_DIM_DONE, Trigger.COUNT)
        u.next_uop = (0, 2, 1)
    return u


class _DveOpPerf(DveOp):
    def compile(self, ver):
        from concourse.dve_ops import get_dve_sub_opcode

        key = getattr(self, "_cached", None)
        if key is not None and key[0] == ver:
            return key[1]
        spec = DveOpSpec(
            name=self.name,
            opcode=get_dve_sub_opcode(self.name),
            uops=[_sd_state_1x(k) for k in ("seed", "steady", "step")],
            uops_2x=[_sd_state_2x(k) for k in ("seed", "steady", "step")],
            perf_max=1,
            rd1_en=True,
        )
        spec.validate(ver)
        object.__setattr__(self, "_cached", (ver, spec))
        return spec


SCAN_DOT = _DveOpPerf("SCAN_DOT_ANT", _SCAN_DOT_SPEC, subdim=False, uops_sha={})


def _register_scan_dot():
    if SCAN_DOT.name in dve_ops_mod._SUB_OPCODE_FOR_NAME:
        return
    dve_ops_mod.OPS.append(SCAN_DOT)
    dve_ops_mod.CUSTOM_DVE_SPECS[SCAN_DOT.name] = SCAN_DOT.spec
    dve_ops_mod._SUB_OPCODE_FOR_NAME[SCAN_DOT.name] = (
        dve_ops_mod._CUSTOM_DVE_ROW_BASE + len(dve_ops_mod.OPS) - 1
    )
    assert dve_ops_mod._SUB_OPCODE_FOR_NAME[SCAN_DOT.name] < 0x20


def _scan_dot_pg(vec, out, in0, in1):
    """Emit the paged SCAN_DOT: in0 [128, S, N] fp16 (pages reset the
    accumulator), in1 [128, S, N] fp16 (broadcast ok), out [128, S*N] fp16."""
    _register_scan_dot()
    op = SCAN_DOT
    if op.name not in vec.bass.m.ant_custom_dve_ops:
        vec.bass.m.ant_custom_dve_ops = sorted(
            {*vec.bass.m.ant_custom_dve_ops, op.name}
        )
    from concourse.dve_ops import get_dve_sub_opcode

    assert_partition_dims_match(out, in0, in1, error_msg_prefix="scan_dot: ")
    in1_3d = len(in1.shape) > 2
    shape = (bass_isa.CustomDveShape.STT if in1_3d
             else bass_isa.CustomDveShape.TTSS)
    isa_opcode = vec.bass.isa.Opcode[
        f"NEURON_ISA_TPB_OPCODE_CUSTOM_DVE_ANT_{shape.slot()}"
    ].value
    ins = [vec.lower_ap(in0, for_isa=True, opt=False),
           vec.lower_ap(in1, for_isa=True, opt=not in1_3d),
           mybir.ImmediateValue(dtype=mybir.dt.float32, value=0.0),
           mybir.ImmediateValue(dtype=mybir.dt.float32, value=0.0)]
    outs = [vec.lower_ap(out, for_isa=True, opt=True)]
    return vec.add_instruction(
        bass_isa.InstCustomDveAnt(
            name=vec.bass.get_next_instruction_name(),
            op_name=op.name,
            rd1_en=True,
            subdim=0x02,
            imm2=0.0,
            shape=shape,
            row=get_dve_sub_opcode(op.name),
            isa_opcode=isa_opcode,
            ins=ins,
            outs=outs,
            perf_max=1,
        )
    )


# --------------------------------------------------------------------------
# Kernel program
# --------------------------------------------------------------------------

def build_program(debug=False, enable_asserts=False, enc_bufs=7,
                  bigp_bufs=3, with_logm=True):
    ng = S // SS              # total groups (256)
    NACC = NPAIR + 6          # pall cols: one per stream unit (22)

    nc = bacc.Bacc("TRN2", target_bir_lowering=False, debug=debug,
                   enable_asserts=enable_asserts, num_devices=NCORES)

    enc = nc.dram_tensor("enc", [NPAIR, 128, PW], F16, kind="ExternalInput").ap()
    w_a = nc.dram_tensor("w_a", [128, 4 * H], F16, kind="ExternalInput").ap()
    w_rT = nc.dram_tensor("w_rT", [128, 8 * H], F16, kind="ExternalInput").ap()
    c32 = nc.dram_tensor("c32", [128, BC + 1], F32, kind="ExternalInput").ap()
    oneh = nc.dram_tensor("oneh", [128, GPP * BC], BF16, kind="ExternalInput").ap()
    c16 = nc.dram_tensor("c16", [128, 4 * BC], F16, kind="ExternalInput").ap()
    p8 = nc.dram_tensor("p8", [BC, H], F32, kind="ExternalInput").ap()
    rm16 = nc.dram_tensor("rm16", [BC, 128], F16, kind="ExternalInput").ap()
    s2m = nc.dram_tensor("s2m", [128, BC], BF16, kind="ExternalInput").ap()
    if with_logm:
        mask_p = nc.dram_tensor("mask_p", [128, ng], I32, kind="ExternalInput").ap()
    out = nc.dram_tensor("out", [BC, H], F32, kind="ExternalOutput").ap()

    with tile.TileContext(nc) as tc:
        with (
            tc.tile_pool(name="const", bufs=1) as cpool,
            tc.tile_pool(name="encp", bufs=enc_bufs) as encp,
            tc.tile_pool(name="bigpp", bufs=bigp_bufs) as bigpp,
            tc.tile_pool(name="smallp", bufs=6) as smallp,
            tc.tile_pool(name="psum", bufs=1, space="PSUM") as pp,
            tc.tile_pool(name="psumtr", bufs=2, space="PSUM") as pptr,
        ):
            w_a_sb = cpool.tile([128, 4 * H], F16)      # [128, (c4, k512)]
            w_rT_sb = cpool.tile([128, 8 * H], F16)     # [128, (c8, n512)]
            c32_sb = cpool.tile([128, BC + 1], F32)     # r_t/8 | -C col
            oneh_sb = cpool.tile([128, GPP * BC], BF16)
            h_tT16_sb = cpool.tile([128, 4 * BC], F16)  # [128, (c4, b8)]
            p8_sb = cpool.tile([BC, H], F32)            # b_r_rep
            rm16_sb = cpool.tile([BC, 128], F16)
            s2_sb = cpool.tile([128, BC], BF16)
            urep_sb = cpool.tile([128, H], F16)
            u_sb = cpool.tile([BC, H], F16)
            pall_sb = cpool.tile([128, NACC], F32)
            pscr_sb = cpool.tile([128, NACC], F32)
            rowsum_sb = cpool.tile([128, 1], F32)
            linv_sb = cpool.tile([BC, 1], F32)
            ct4_sb = cpool.tile([128, H], BF16)
            catT_sb = cpool.tile([128, 4 * BC], BF16)
            oh_sb = cpool.tile([BC, H], F32)
            o2_sb = cpool.tile([BC, H], F32)
            out_sb = cpool.tile([BC, H], F32)
            if with_logm:
                mask_sb = cpool.tile([128, ng], I32)
                maskf_sb = cpool.tile([128, ng], F32)
                logm_sb = cpool.tile([128, ng], F32)

            rT_sb = c32_sb[:, 0:BC]
            negC_sb = c32_sb[:, BC:BC + 1]
            brr_sb = p8_sb

            # --- all weights on the fast HW queues, split across both,
            # ahead of the enc stream: same total bytes, but nothing on the
            # PE can head-of-line block on a slow SWDGE transfer (the tile
            # scheduler freely reorders the setup matmuls) ---
            nc.sync.dma_start(w_a_sb[:, :2 * H], w_a[:, :2 * H])
            nc.scalar.dma_start(w_a_sb[:, 2 * H:], w_a[:, 2 * H:])
            nc.sync.dma_start(c32_sb[:], c32[:])
            nc.sync.dma_start(rm16_sb[:], rm16[:])
            nc.scalar.dma_start(h_tT16_sb[:], c16[:])
            nc.sync.dma_start(p8_sb[:], p8[:])
            nc.sync.dma_start(w_rT_sb[:, :4 * H], w_rT[:, :4 * H])
            nc.scalar.dma_start(w_rT_sb[:, 4 * H:], w_rT[:, 4 * H:])
            nc.gpsimd.dma_start(oneh_sb[:], oneh[:])
            nc.gpsimd.dma_start(s2_sb[:], s2m[:])
            if with_logm:
                nc.gpsimd.dma_start(mask_sb[:], mask_p[:])

            # u = h_t @ W_a  (contraction over h, 4 chunks of 128)
            psum_u = pp.tile([BC, H], F32)
            for c in range(4):
                nc.tensor.matmul(psum_u[:], h_tT16_sb[:, c * BC:(c + 1) * BC],
                                 w_a_sb[:, c * H:(c + 1) * H],
                                 start=(c == 0), stop=(c == 3))
            nc.scalar.copy(u_sb[:], psum_u[:])

            # u_rep[p, h] = u[p % BC, h] via R[b, p] = (p % BC == b); fp16 out
            psum_ur = pp.tile([128, H], F32)
            nc.tensor.matmul(psum_ur[:], rm16_sb[:], u_sb[:], start=True, stop=True)
            nc.vector.tensor_copy(urep_sb[:], psum_ur[:])

            if with_logm:
                nc.vector.tensor_copy(maskf_sb[:], mask_sb[:])
                nc.scalar.activation(logm_sb[:], maskf_sb[:], AF.Ln)

            psum_oh = pp.tile([BC, H], F32)

            def emit_oh():
                # h_t half of the output projection; emitted mid-loop so the
                # w_rT dependency can't block the startup u-chain on the PE.
                for ic in range(4):
                    nc.tensor.matmul(psum_oh[:],
                                     h_tT16_sb[:, ic * BC:(ic + 1) * BC],
                                     w_rT_sb[:, (ic + 4) * H:(ic + 5) * H],
                                     start=(ic == 0), stop=(ic == 3))
                nc.vector.tensor_add(oh_sb[:], psum_oh[:], brr_sb[:])

            psum_oc = pp.tile([BC, H], F32)
            psum_ct4 = pp.tile([128, H], F32)
            psum_l = pp.tile([BC, 1], F32)

            # stream units: first two pairs as single tiles (compute can
            # start during the DMA ramp), middle as full pairs, last pair
            # as quarters (drain overlaps the stream tail).
            units = []
            for t in range(NPAIR):
                if t in (0, NPAIR - 1):
                    units += [(t, q * (GPP // 4), GPP // 4) for q in range(4)]
                else:
                    units += [(t, 0, GPP)]
            NUNIT = len(units)
            LOOKAHEAD = 5
            enc_sbs = {}

            def emit_unit_dma(i):
                t, gbase, nu = units[i]
                w = nu * H
                enc_sb = encp.tile([128, w], F16)
                src = enc[t, :, gbase * H:gbase * H + w]
                hw = w // 2
                nc.sync.dma_start(enc_sb[:, :hw], src[:, :hw])
                nc.scalar.dma_start(enc_sb[:, hw:], src[:, hw:])
                enc_sbs[i] = enc_sb

            def emit_mult_mms(work):
                # DVE psp = expall * onehot (bf16, 2x) + the PE matmul batch.
                # Called one unit late so the in-order DVE never bubbles on
                # the Pool->ACT roundtrip after its own scan.
                t, gbase, nu, enc_sb, expall, psp = work
                nc.vector.tensor_mul(
                    psp[:], expall[:], oneh_sb[:, :nu * BC])
                psp3 = psp[:].rearrange("p (j b) -> p j b", b=BC)
                # Alternate PE column-groups so each LDWEIGHTS targets a
                # different 32-strip than the in-flight matmul.
                for jl in range(nu):
                    j = gbase + jl
                    jj = j % 2
                    nc.tensor.matmul(psum_ct4[32 * jj:32 * jj + BC, :],
                                     psp3[:, jl, :],
                                     enc_sb[:, jl * H:(jl + 1) * H],
                                     start=(t == 0 and j < 2),
                                     stop=(t == NPAIR - 1 and j >= GPP - 2),
                                     tile_position=(0, 32 * jj),
                                     skip_group_check=True)

            def emit_unit(i, acc_col):
                """Scan + score/exp chain for unit i = groups
                [gbase, gbase+nu) of pair t.  The mult+matmul batch goes
directly chained."""
                t, gbase, nu = units[i]
                enc_sb = enc_sbs.pop(i)
                bigp = bigpp.tile([128, nu * H], F16)
                scores0 = smallp.tile([128, nu], F32)
                scores = smallp.tile([128, nu], F32)
                expall = smallp.tile([128, nu * BC], BF16)
                psp = smallp.tile([128, nu * BC], BF16)
                # paged scan-dot: page g's final (coarse, resid) pair lands
                # at cols g*H+510 / g*H+511.  (DVE: scans only.)
                _scan_dot_pg(nc.vector, bigp[:],
                             enc_sb[:].rearrange("p (g n) -> p g n", g=nu),
                             urep_sb[:].rearrange("p (o n) -> p o n", o=1)
                             .broadcast_to([128, nu, H]))
                # previous unit's mult+matmuls, now that its expall is done
                while pending:
                    emit_mult_mms(pending.pop(0))
                # page-sum extraction on Pool (small strided add — cheap)
                nc.gpsimd.tensor_add(
                    scores0[:] if with_logm else scores[:],
                    bigp[:].rearrange("p (g n) -> p g n", g=nu)[:, :, H - 2],
                    bigp[:].rearrange("p (g n) -> p g n", g=nu)[:, :, H - 1])
                if with_logm:
                    nc.gpsimd.tensor_add(
                        scores[:], scores0[:],
                        logm_sb[:, t * GPP + gbase:t * GPP + gbase + nu])
                # ACT reads the scores BROADCAST over b (8 dup cols — accum
                # is 8x the partial denominator; folded into r_t as a
                # host-side 1/8), then DVE zeroes the off-diagonal with a
                # 2x bf16 mult (one unit of skew via `pending`).
                nc.scalar.activation(
                    expall[:].rearrange("p (j b) -> p j b", b=BC),
                    scores[:].rearrange("p (j o) -> p j o", o=1)
                    .broadcast_to([128, nu, BC]),
                    AF.Exp, bias=negC_sb,
                    accum_out=pall_sb[:, acc_col:acc_col + 1])
                pending.append((t, gbase, nu, enc_sb, expall, psp))

            emit_oh()

            pending = []
            for i in range(LOOKAHEAD):
                emit_unit_dma(i)
            for i in range(NUNIT):
                # keep the scalar engine's DMA issues ahead of its exps —
                # the engine is in-order, so an exp emitted before a
                # dma_start would gate the stream on the compute pipeline.
                if i + LOOKAHEAD < NUNIT:
                    emit_unit_dma(i + LOOKAHEAD)
                emit_unit(i, i)
            while pending:
                emit_mult_mms(pending.pop(0))

            # ---- epilogue ----
            # denominator: rowsum over pall -> l per batch -> 1/l
            nc.scalar.activation(pscr_sb[:], pall_sb[:], AF.Copy,
                                 accum_out=rowsum_sb[:])
            nc.tensor.matmul(psum_l[:], rT_sb[:], rowsum_sb[:],
                             start=True, stop=True)
            nc.vector.reciprocal(linv_sb[:], psum_l[:])
            # unnormalized c_t: merge the two PE row-groups and transpose in
            # one matmul per 128-chunk of h (stationary ct4 chunk x s2).
            nc.scalar.copy(ct4_sb[:], psum_ct4[:])
            for hc in range(4):
                ptr = pptr.tile([128, BC], F32)
                nc.tensor.matmul(ptr[:], ct4_sb[:, hc * 128:(hc + 1) * 128],
                                 s2_sb[:], start=True, stop=True)
                nc.scalar.copy(catT_sb[:, hc * BC:(hc + 1) * BC], ptr[:])
            for ic in range(4):
                nc.tensor.matmul(psum_oc[:], catT_sb[:, ic * BC:(ic + 1) * BC],
                                 w_rT_sb[:, ic * H:(ic + 1) * H],
                                 start=(ic == 0), stop=(ic == 3))
            # out = tanh(oc * (1/l) + oh)
            nc.vector.scalar_tensor_tensor(o2_sb[:], psum_oc[:], linv_sb[:],
                                           oh_sb[:], ALU.mult, ALU.add)
            nc.scalar.activation(out_sb[:], o2_sb[:], AF.Tanh)
            nc.sync.dma_start(out[:], out_sb[:])

    nc.compile()
    return nc


def prep_in_maps(inputs):
    enc = np.asarray(inputs["encoder_hidden_states"]).astype(np.float32, copy=False)
    h_t = np.asarray(inputs["h_t"]).astype(np.float32, copy=False)
    mask = np.asarray(inputs["encoder_context_mask"]).astype(np.int32, copy=False)
    W_a = np.asarray(inputs["W_a"]).astype(np.float32, copy=False)
    W_r = np.asarray(inputs["W_r"]).astype(np.float32, copy=False)
    b_r = np.asarray(inputs["b_r"]).astype(np.float32, copy=False)

    ng = S // SS
    p_idx = np.arange(128)
    b_idx = np.arange(BC)
    # w_a packed [128, (c4, k512)] fp16
    w_a_p = np.ascontiguousarray(
        W_a.reshape(4, 128, H).transpose(1, 0, 2).reshape(128, 4 * H)
        .astype(np.float16))
    # w_rT packed [128, (c8, n512)] fp16
    w_rT_p = np.ascontiguousarray(
        W_r.T.reshape(8, 128, H).transpose(1, 0, 2).reshape(128, 8 * H)
        .astype(np.float16))
    r_mat = (p_idx[None, :] % BC == b_idx[:, None]).astype(np.float32)
    # the expall accum double-counts each partial denominator 8x (one per
    # broadcast dup column) — fold the 1/8 into the collapse matrix
    c32 = np.ascontiguousarray(np.concatenate(
        [r_mat.T / 8.0, np.full((128, 1), -C_SHIFT, np.float32)], axis=1))
    onehot = (p_idx[:, None] % BC == b_idx[None, :]).astype(np.float32)
    oneh_full = np.ascontiguousarray(
        np.tile(onehot, (1, GPP)).astype(ml_dtypes.bfloat16))
    s2_mat_np = np.zeros((128, BC), np.float32)
    for b in range(BC):
        s2_mat_np[b, b] = 1.0
        s2_mat_np[32 + b, b] = 1.0
    b_r_rep = np.broadcast_to(b_r, (BC, H)).astype(np.float32)
    p8 = np.ascontiguousarray(b_r_rep)
    rm16 = np.ascontiguousarray(r_mat.astype(np.float16))

    in_maps = []
    for c in range(NCORES):
        bs = slice(c * BC, (c + 1) * BC)
        mask_c = mask[bs, :]
        mask_p = np.ascontiguousarray(
            mask_c.reshape(BC, ng, SS).transpose(2, 0, 1).reshape(128, ng))
        h_c = h_t[bs]                                   # [BC, H]
        c16 = np.ascontiguousarray(
            h_c.T.reshape(4, 128, BC).transpose(1, 0, 2).reshape(128, 4 * BC)
            .astype(np.float16))
        in_maps.append({
            "enc": np.ascontiguousarray(
                enc[:, bs, :]
                .reshape(NPAIR, 2 * (S_TILE // SS), SS, BC, H)
                .transpose(0, 2, 3, 1, 4)
                .reshape(NPAIR, 128, PW)
                .astype(np.float16)),
            "w_a": w_a_p,
            "w_rT": w_rT_p,
            "c32": c32,
            "c16": c16,
            "p8": p8,
            "rm16": rm16,
            "s2m": s2_mat_np.astype(ml_dtypes.bfloat16),
            "oneh": oneh_full,
            "mask_p": mask_p,
        })
    return in_maps


_CACHE = {}


def _reset_device():
    # Best-effort recovery of a wedged NeuronCore left by a previous process.
    try:
        import ctypes
        lib = ctypes.CDLL("/opt/axon/libaxon_pjrt.so")
        lib.axon_reset.restype = ctypes.c_int64
        import jax
        jax.devices()
        lib.axon_reset()
    except Exception:
        pass


def run(inputs, trace=False, **kw):
    mask = np.asarray(inputs["encoder_context_mask"])
    with_logm = not bool((mask == 1).all())
    key = ("nc", with_logm)
    if key not in _CACHE:
        _CACHE[key] = build_program(with_logm=with_logm)
    nc = _CACHE[key]
    in_maps = prep_in_maps(inputs)
    if not with_logm:
        for m in in_maps:
            m.pop("mask_p", None)

    def _once():
        try:
            return run_bass_kernel_spmd(nc, in_maps, list(range(NCORES)),
                                        trace=trace, **kw)
        except Exception:
            _reset_device()
            return run_bass_kernel_spmd(nc, in_maps, list(range(NCORES)),
                                        trace=trace, **kw)

    # Rare timing-dependent glitch under heavy device throttling can corrupt
    # a run (NaN / out-of-range tanh output). Detect and retry.
    for attempt in range(3):
        res = _once()
        full = np.concatenate([np.asarray(res.results[c]["out"])
                               for c in range(NCORES)], axis=0).astype(np.float32)
        if np.isfinite(full).all() and np.abs(full).max() <= 1.0 + 1e-3:
            break
    return full, res


def kernel(**inputs):
    return run(inputs)[0]


# revision 29
# speedup vs baseline: 1.1403x; 1.1403x over previous
"""Trainium2 Bass kernel for the Luong-attention layer (nn_AttentionLayer).

Math (reference):
    hs_proj = enc @ W_a.T + b_a                  # [S,B,H]
    scores[s,b] = hs_proj[s,b] . h_t[b]          # [S,B]
    scores += log(mask).T
    a = softmax(scores, axis=0)
    c_t[b] = sum_s a[s,b] * enc[s,b]             # [B,H]
    out = tanh([c_t, h_t] @ W_r.T + b_r)         # [B,H]

Restructuring:
  * scores[s,b] = enc[s,b] . u[b] with u = h_t @ W_a (b_a cancels in the
    s-axis softmax); softmax shift folded as a fixed constant C=40.
  * Data-parallel over batch: 8 cores x 8 batches, no collectives. Each
    core streams its enc shard ONCE from HBM as fp16 (32 MiB).
  * enc streamed in PAIRS of 128-row s-tiles (2 MiB per pair), the two
    1 MiB halves on the sync and scalar HW DMA queues so both queues
    saturate from the start; all weights/constants ride the gpsimd
    SWDGE queue packed into a handful of contiguous transfers.
  * Scores on DVE via the custom paged "SCAN_DOT" op (2 fp16
    elems/lane/cycle, compensated coarse/resid fp16 output pair per
    page) — ONE instruction per pair (16 pages), nothing else on DVE in
    steady state so the DVE (0.96 GHz) keeps pace with the 425 GB/s DMA
    stream.
  * The two small adds (page-sum extraction + diagonal-mask spread) run
    on the otherwise-idle Pool (gpsimd) engine; exp on ACT with
    accumulated denominators.
  * PE: psum += psp(bf16).T @ enc(fp16), alternating between two PE
    column-groups (tile_position) so LDWEIGHTS hides under the in-flight
    matmul; the final merge of the two accumulator row-sets is fused
    into the output transposes (stationary ct4 chunk x selector s2).
  * Epilogue: 1/l scaling deferred past the output projection and fused
    with the +oh add in one scalar_tensor_tensor; the last pair's
    exp/matmuls are split into quarters to shorten the post-DMA drain.
Per-core: partitions p = (s_sub 16, b 8), h on the free axis; enc
host-pre-permuted so each 1 MiB fp16 half-pair is one contiguous DMA.
"""

import sys

if "/opt/trn_rl_repo" not in sys.path:
    sys.path.insert(0, "/opt/trn_rl_repo")

import ml_dtypes
import numpy as np

import concourse.bacc as bacc
import concourse.dve_ops as dve_ops_mod
import concourse.mybir as mybir
from concourse import bass_isa, tile
from concourse.bass import assert_partition_dims_match
from concourse.bass_utils import run_bass_kernel_spmd
from concourse.dve_ops import DveOp
from concourse.dve_spec import C0, Spec, Src0, Src1, lower as dve_lower, scan
from concourse.dve_uop import (
    AluInp,
    AluOp as DveAluOp,
    DelayInp,
    DveOpSpec,
    InpSel,
    OutPath,
    OutSel,
    Trigger,
    UopConfig,
)

S, B, H = 4096, 64, 512
NCORES = 8
BC = B // NCORES          # 8 batches per core
SS = 128 // BC            # 16 s-positions per group
S_TILE = 128              # s-positions per tile
NPAIR = S // (2 * S_TILE)  # 16 DMA pairs (2 tiles each)
GPP = 2 * (S_TILE // SS)  # 16 groups per pair
PW = GPP * H              # pair width in elements (8192)
C_SHIFT = 40.0
NEG_INF = -1.0e30
F32 = mybir.dt.float32
F16 = mybir.dt.float16
BF16 = mybir.dt.bfloat16
I32 = mybir.dt.int32
AF = mybir.ActivationFunctionType
ALU = mybir.AluOpType

# --------------------------------------------------------------------------
# SCAN_DOT custom DVE op: fused fp16 mul + fp32 accumulate at 2 elems/cycle,
# emitting the running sum as a compensated (coarse, resid) fp16 pair.
# --------------------------------------------------------------------------

_PD = [AluInp.PREV_DELAY_0, AluInp.PREV_DELAY_1, AluInp.PREV_DELAY_2,
       AluInp.PREV_DELAY_3, AluInp.PREV_DELAY_4, AluInp.PREV_DELAY_5]

_SCAN_DOT_SPEC = Spec(
    body=scan(DveAluOp.ADD, Src0 * Src1, init=C0),
    reference=lambda in0, in1, s0, s1, imm2: (
        np.cumsum(in0.astype(np.float32) * in1.astype(np.float32), axis=-1) + s0
    ),
)


def _sd_inputs(u, two_x):
    u.enable_input(InpSel.SRC_0, 1)
    u.enable_input(InpSel.SRC_1, 2)
    if two_x:
        u.enable_input(InpSel.SRC_0_HI, 3)
        u.enable_input(InpSel.SRC_1_HI, 4)
    u.enable_input(InpSel.CONST_0, 5)
    u.enable_input(InpSel.MASK16_SL16, 6)


def _sd_state_2x(kind):
    """kind: 'seed' | 'steady' | 'step' (paged, 3-state FSM)."""
    u = UopConfig()
    _sd_inputs(u, two_x=True)
    dp = u.datapath_config
    if kind == "seed":
        dp[0].pass_through_alu().pass_through_delay(4)
        dp[1].pass_through_alu().pass_through_delay(4)
        dp[2].pass_through_alu().pass_through_delay(4)
        dp[3].enable_alu(DveAluOp.BYPASS, _PD[4], _PD[4])
        for i in range(4, 8):
            dp[i].pass_through_alu()
        u.repeat_count = 1
        u.trigger = (Trigger.COUNT, Trigger.NONE, Trigger.NONE)
        u.next_uop = (1, 0, 0)
        return u
    dp[0].enable_alu(DveAluOp.MULTIPLY, _PD[0], _PD[1]).pass_through_delay(2, 3, 5)
    dp[1].enable_alu(DveAluOp.MULTIPLY, _PD[2], _PD[3]).pass_through_delay(5)
    dp[1].enable_delay_from_src(DelayInp.PREV_ALU_OUT, 0)
    dp[2].enable_alu(DveAluOp.ADD, AluInp.PREV_ALU_OUT, _PD[0]).pass_through_delay(5)
    if kind == "steady":
        dp[3].enable_alu(DveAluOp.ADD, AluInp.CURR_ALU_OUT, AluInp.PREV_ALU_OUT)
    else:  # step: first pair of a new page -> acc = 0 + pair
        dp[3].enable_alu(DveAluOp.BYPASS, AluInp.PREV_ALU_OUT, AluInp.PREV_ALU_OUT)
    dp[3].pass_through_delay(5)
    dp[4].enable_alu(DveAluOp.BITWISE_AND, AluInp.PREV_ALU_OUT, _PD[5])
    dp[4].enable_delay_from_src(DelayInp.PREV_ALU_OUT, 0)
    dp[5].enable_alu(DveAluOp.SUBTRACT, _PD[0], AluInp.PREV_ALU_OUT)
    dp[5].enable_delay_from_src(DelayInp.PREV_ALU_OUT, 1)
    dp[6].pass_through_alu().pass_through_delay(1)
    dp[7].pass_through_alu().pass_through_delay(1)
    u.enable_output(OutSel.DELAY_1, OutPath.WR0_LO)   # coarse -> even col
    u.enable_output(OutSel.ALU_OUT, OutPath.WR0_HI)   # resid  -> odd col
    u.require_inp0 = 1
    u.require_inp1 = 1
    if kind == "steady":
        u.trigger = (Trigger.SRC_TENSOR_DONE, Trigger.SUB_DIM_DONE, Trigger.NONE)
        u.next_uop = (0, 2, 0)
    else:
        u.repeat_count = 1
        u.trigger = (Trigger.SRC_TENSOR_DONE, Trigger.SUB_DIM_DONE, Trigger.COUNT)
        u.next_uop = (0, 2, 1)
    return u


def _sd_state_1x(kind):
    """1x fallback twin (one elem/cycle, plain prefix per page; the last
    column of each page is the full page sum, col N-2 is prefix N-1 — the
    coarse+resid read degrades, so call sites must qualify for 2x; a 1x
    fallback is caught by the rel-err gate)."""
    u = UopConfig()
    _sd_inputs(u, two_x=False)
    dp = u.datapath_config
    if kind == "seed":
        dp[0].pass_through_alu().pass_through_delay(4)
        dp[1].pass_through_alu().pass_through_delay(4)
        dp[2].pass_through_alu().pass_through_delay(4)
        dp[3].enable_alu(DveAluOp.BYPASS, _PD[4], _PD[4])
        for i in range(4, 8):
            dp[i].pass_through_alu()
        u.repeat_count = 1
        u.trigger = (Trigger.COUNT, Trigger.NONE, Trigger.NONE)
        u.next_uop = (1, 0, 0)
        return u
    dp[0].enable_alu(DveAluOp.MULTIPLY, _PD[0], _PD[1])
    dp[1].pass_through_alu()
    dp[2].pass_through_alu()
    if kind == "steady":
        dp[3].enable_alu(DveAluOp.ADD, AluInp.CURR_ALU_OUT, AluInp.PREV_ALU_OUT)
    else:
        dp[3].enable_alu(DveAluOp.BYPASS, AluInp.PREV_ALU_OUT, AluInp.PREV_ALU_OUT)
    for i in range(4, 8):
        dp[i].pass_through_alu()
    u.enable_output(OutSel.ALU_OUT, OutPath.WR0_LO)
    u.require_inp0 = 1
    u.require_inp1 = 1
    if kind == "steady":
        u.trigger = (Trigger.SRC_TENSOR_DONE, Trigger.SUB_DIM_DONE, Trigger.NONE)
        u.next_uop = (0, 2, 0)
    else:
        u.repeat_count = 1
        u.trigger = (Trigger.SRC_TENSOR_DONE, Trigger.SUB_DIM_DONE, Trigger.COUNT)
        u.next_uop = (0, 2, 1)
    return u


class _DveOpPerf(DveOp):
    def compile(self, ver):
        from concourse.dve_ops import get_dve_sub_opcode

        key = getattr(self, "_cached", None)
        if key is not None and key[0] == ver:
            return key[1]
        spec = DveOpSpec(
            name=self.name,
            opcode=get_dve_sub_opcode(self.name),
            uops=[_sd_state_1x(k) for k in ("seed", "steady", "step")],
            uops_2x=[_sd_state_2x(k) for k in ("seed", "steady", "step")],
            perf_max=1,
            rd1_en=True,
        )
        spec.validate(ver)
        object.__setattr__(self, "_cached", (ver, spec))
        return spec


SCAN_DOT = _DveOpPerf("SCAN_DOT_ANT", _SCAN_DOT_SPEC, subdim=False, uops_sha={})


def _register_scan_dot():
    if SCAN_DOT.name in dve_ops_mod._SUB_OPCODE_FOR_NAME:
        return
    dve_ops_mod.OPS.append(SCAN_DOT)
    dve_ops_mod.CUSTOM_DVE_SPECS[SCAN_DOT.name] = SCAN_DOT.spec
    dve_ops_mod._SUB_OPCODE_FOR_NAME[SCAN_DOT.name] = (
        dve_ops_mod._CUSTOM_DVE_ROW_BASE + len(dve_ops_mod.OPS) - 1
    )
    assert dve_ops_mod._SUB_OPCODE_FOR_NAME[SCAN_DOT.name] < 0x20


def _scan_dot_pg(vec, out, in0, in1):
    """Emit the paged SCAN_DOT: in0 [128, S, N] fp16 (pages reset the
    accumulator), in1 [128, S, N] fp16 (broadcast ok), out [128, S*N] fp16."""
    _register_scan_dot()
    op = SCAN_DOT
    if op.name not in vec.bass.m.ant_custom_dve_ops:
        vec.bass.m.ant_custom_dve_ops = sorted(
            {*vec.bass.m.ant_custom_dve_ops, op.name}
        )
    from concourse.dve_ops import get_dve_sub_opcode

    assert_partition_dims_match(out, in0, in1, error_msg_prefix="scan_dot: ")
    in1_3d = len(in1.shape) > 2
    shape = (bass_isa.CustomDveShape.STT if in1_3d
             else bass_isa.CustomDveShape.TTSS)
    isa_opcode = vec.bass.isa.Opcode[
        f"NEURON_ISA_TPB_OPCODE_CUSTOM_DVE_ANT_{shape.slot()}"
    ].value
    ins = [vec.lower_ap(in0, for_isa=True, opt=False),
           vec.lower_ap(in1, for_isa=True, opt=not in1_3d),
           mybir.ImmediateValue(dtype=mybir.dt.float32, value=0.0),
           mybir.ImmediateValue(dtype=mybir.dt.float32, value=0.0)]
    outs = [vec.lower_ap(out, for_isa=True, opt=True)]
    return vec.add_instruction(
        bass_isa.InstCustomDveAnt(
            name=vec.bass.get_next_instruction_name(),
            op_name=op.name,
            rd1_en=True,
            subdim=0x02,
            imm2=0.0,
            shape=shape,
            row=get_dve_sub_opcode(op.name),
            isa_opcode=isa_opcode,
            ins=ins,
            outs=outs,
            perf_max=1,
        )
    )


# --------------------------------------------------------------------------
# Kernel program
# --------------------------------------------------------------------------

def build_program(debug=False, enable_asserts=False, enc_bufs=7,
                  bigp_bufs=3, with_logm=True):
    ng = S // SS              # total groups (256)
    NACC = NPAIR + 6          # pall cols: one per stream unit (22)

    nc = bacc.Bacc("TRN2", target_bir_lowering=False, debug=debug,
                   enable_asserts=enable_asserts, num_devices=NCORES)

    enc = nc.dram_tensor("enc", [NPAIR, 128, PW], F16, kind="ExternalInput").ap()
    w_a = nc.dram_tensor("w_a", [128, 4 * H], F16, kind="ExternalInput").ap()
    w_rT = nc.dram_tensor("w_rT", [128, 8 * H], F16, kind="ExternalInput").ap()
    c32 = nc.dram_tensor("c32", [128, BC + 1], F32, kind="ExternalInput").ap()
    oneh = nc.dram_tensor("oneh", [128, GPP * BC], BF16, kind="ExternalInput").ap()
    c16 = nc.dram_tensor("c16", [128, 4 * BC], F16, kind="ExternalInput").ap()
    p8 = nc.dram_tensor("p8", [BC, H], F32, kind="ExternalInput").ap()
    rm16 = nc.dram_tensor("rm16", [BC, 128], F16, kind="ExternalInput").ap()
    s2m = nc.dram_tensor("s2m", [128, BC], BF16, kind="ExternalInput").ap()
    if with_logm:
        mask_p = nc.dram_tensor("mask_p", [128, ng], I32, kind="ExternalInput").ap()
    out = nc.dram_tensor("out", [BC, H], F32, kind="ExternalOutput").ap()

    with tile.TileContext(nc) as tc:
        with (
            tc.tile_pool(name="const", bufs=1) as cpool,
            tc.tile_pool(name="encp", bufs=enc_bufs) as encp,
            tc.tile_pool(name="bigpp", bufs=bigp_bufs) as bigpp,
            tc.tile_pool(name="smallp", bufs=6) as smallp,
            tc.tile_pool(name="psum", bufs=1, space="PSUM") as pp,
            tc.tile_pool(name="psumtr", bufs=2, space="PSUM") as pptr,
        ):
            w_a_sb = cpool.tile([128, 4 * H], F16)      # [128, (c4, k512)]
            w_rT_sb = cpool.tile([128, 8 * H], F16)     # [128, (c8, n512)]
            c32_sb = cpool.tile([128, BC + 1], F32)     # r_t/8 | -C col
            oneh_sb = cpool.tile([128, GPP * BC], BF16)
            h_tT16_sb = cpool.tile([128, 4 * BC], F16)  # [128, (c4, b8)]
            p8_sb = cpool.tile([BC, H], F32)            # b_r_rep
            rm16_sb = cpool.tile([BC, 128], F16)
            s2_sb = cpool.tile([128, BC], BF16)
            urep_sb = cpool.tile([128, H], F16)
            u_sb = cpool.tile([BC, H], F16)
            pall_sb = cpool.tile([128, NACC], F32)
            pscr_sb = cpool.tile([128, NACC], F32)
            rowsum_sb = cpool.tile([128, 1], F32)
            linv_sb = cpool.tile([BC, 1], F32)
            ct4_sb = cpool.tile([128, H], BF16)
            catT_sb = cpool.tile([128, 4 * BC], BF16)
            oh_sb = cpool.tile([BC, H], F32)
            o2_sb = cpool.tile([BC, H], F32)
            out_sb = cpool.tile([BC, H], F32)
            if with_logm:
                mask_sb = cpool.tile([128, ng], I32)
                maskf_sb = cpool.tile([128, ng], F32)
                logm_sb = cpool.tile([128, ng], F32)

            rT_sb = c32_sb[:, 0:BC]
            negC_sb = c32_sb[:, BC:BC + 1]
            brr_sb = p8_sb

            # --- all weights on the fast HW queues, split across both,
            # ahead of the enc stream: same total bytes, but nothing on the
            # PE can head-of-line block on a slow SWDGE transfer (the tile
            # scheduler freely reorders the setup matmuls) ---
            nc.sync.dma_start(w_a_sb[:, :2 * H], w_a[:, :2 * H])
            nc.scalar.dma_start(w_a_sb[:, 2 * H:], w_a[:, 2 * H:])
            nc.sync.dma_start(c32_sb[:], c32[:])
            nc.sync.dma_start(rm16_sb[:], rm16[:])
            nc.scalar.dma_start(h_tT16_sb[:], c16[:])
            nc.sync.dma_start(p8_sb[:], p8[:])
            nc.sync.dma_start(w_rT_sb[:, :4 * H], w_rT[:, :4 * H])
            nc.scalar.dma_start(w_rT_sb[:, 4 * H:], w_rT[:, 4 * H:])
            nc.gpsimd.dma_start(oneh_sb[:], oneh[:])
            nc.gpsimd.dma_start(s2_sb[:], s2m[:])
            if with_logm:
                nc.gpsimd.dma_start(mask_sb[:], mask_p[:])

            # u = h_t @ W_a  (contraction over h, 4 chunks of 128)
            psum_u = pp.tile([BC, H], F32)
            for c in range(4):
                nc.tensor.matmul(psum_u[:], h_tT16_sb[:, c * BC:(c + 1) * BC],
                                 w_a_sb[:, c * H:(c + 1) * H],
                                 start=(c == 0), stop=(c == 3))
            nc.scalar.copy(u_sb[:], psum_u[:])

            # u_rep[p, h] = u[p % BC, h] via R[b, p] = (p % BC == b); fp16 out
            psum_ur = pp.tile([128, H], F32)
            nc.tensor.matmul(psum_ur[:], rm16_sb[:], u_sb[:], start=True, stop=True)
            nc.vector.tensor_copy(urep_sb[:], psum_ur[:])

            if with_logm:
                nc.vector.tensor_copy(maskf_sb[:], mask_sb[:])
                nc.scalar.activation(logm_sb[:], maskf_sb[:], AF.Ln)

            psum_oh = pp.tile([BC, H], F32)

            def emit_oh():
                # h_t half of the output projection; emitted mid-loop so the
                # w_rT dependency can't block the startup u-chain on the PE.
                for ic in range(4):
                    nc.tensor.matmul(psum_oh[:],
                                     h_tT16_sb[:, ic * BC:(ic + 1) * BC],
                                     w_rT_sb[:, (ic + 4) * H:(ic + 5) * H],
                                     start=(ic == 0), stop=(ic == 3))
                nc.vector.tensor_add(oh_sb[:], psum_oh[:], brr_sb[:])

            psum_oc = pp.tile([BC, H], F32)
            psum_ct4 = pp.tile([128, H], F32)
            psum_l = pp.tile([BC, 1], F32)

            # stream units: first two pairs as single tiles (compute can
            # start during the DMA ramp), middle as full pairs, last pair
            # as quarters (drain overlaps the stream tail).
            units = []
            for t in range(NPAIR):
                if t == NPAIR - 1:
                    units += [(t, q * (GPP // 4), GPP // 4) for q in range(4)]
                else:
                    units += [(t, 0, GPP)]
            NUNIT = len(units)
            LOOKAHEAD = 5
            enc_sbs = {}

            def emit_unit_dma(i):
                t, gbase, nu = units[i]
                w = nu * H
                enc_sb = encp.tile([128, w], F16)
                src = enc[t, :, gbase * H:gbase * H + w]
                hw = w // 2
                nc.sync.dma_start(enc_sb[:, :hw], src[:, :hw])
                nc.scalar.dma_start(enc_sb[:, hw:], src[:, hw:])
                enc_sbs[i] = enc_sb

            def emit_mult_mms(work):
                # DVE psp = expall * onehot (bf16, 2x) + the PE matmul batch.
                # Called one unit late so the in-order DVE never bubbles on
                # the Pool->ACT roundtrip after its own scan.
                t, gbase, nu, enc_sb, expall, psp = work
                nc.vector.tensor_mul(
                    psp[:], expall[:], oneh_sb[:, :nu * BC])
                psp3 = psp[:].rearrange("p (j b) -> p j b", b=BC)
                # Alternate PE column-groups so each LDWEIGHTS targets a
                # different 32-strip than the in-flight matmul.
                for jl in range(nu):
                    j = gbase + jl
                    jj = j % 2
                    nc.tensor.matmul(psum_ct4[32 * jj:32 * jj + BC, :],
                                     psp3[:, jl, :],
                                     enc_sb[:, jl * H:(jl + 1) * H],
                                     start=(t == 0 and j < 2),
                                     stop=(t == NPAIR - 1 and j >= GPP - 2),
                                     tile_position=(0, 32 * jj),
                                     skip_group_check=True)

            def emit_unit(i, acc_col):
                """Scan + score/exp chain for unit i = groups
                [gbase, gbase+nu) of pair t.  The mult+matmul batch goes
directly chained."""
                t, gbase, nu = units[i]
                enc_sb = enc_sbs.pop(i)
                bigp = bigpp.tile([128, nu * H], F16)
                scores0 = smallp.tile([128, nu], F32)
                scores = smallp.tile([128, nu], F32)
                expall = smallp.tile([128, nu * BC], BF16)
                psp = smallp.tile([128, nu * BC], BF16)
                # paged scan-dot: page g's final (coarse, resid) pair lands
                # at cols g*H+510 / g*H+511.  (DVE: scans only.)
                _scan_dot_pg(nc.vector, bigp[:],
                             enc_sb[:].rearrange("p (g n) -> p g n", g=nu),
                             urep_sb[:].rearrange("p (o n) -> p o n", o=1)
                             .broadcast_to([128, nu, H]))
                # previous unit's mult+matmuls, now that its expall is done
                while pending:
                    emit_mult_mms(pending.pop(0))
                # page-sum extraction on Pool (small strided add — cheap)
                nc.gpsimd.tensor_add(
                    scores0[:] if with_logm else scores[:],
                    bigp[:].rearrange("p (g n) -> p g n", g=nu)[:, :, H - 2],
                    bigp[:].rearrange("p (g n) -> p g n", g=nu)[:, :, H - 1])
                if with_logm:
                    nc.gpsimd.tensor_add(
                        scores[:], scores0[:],
                        logm_sb[:, t * GPP + gbase:t * GPP + gbase + nu])
                # ACT reads the scores BROADCAST over b (8 dup cols — accum
                # is 8x the partial denominator; folded into r_t as a
                # host-side 1/8), then DVE zeroes the off-diagonal with a
                # 2x bf16 mult (one unit of skew via `pending`).
                nc.scalar.activation(
                    expall[:].rearrange("p (j b) -> p j b", b=BC),
                    scores[:].rearrange("p (j o) -> p j o", o=1)
                    .broadcast_to([128, nu, BC]),
                    AF.Exp, bias=negC_sb,
                    accum_out=pall_sb[:, acc_col:acc_col + 1])
                pending.append((t, gbase, nu, enc_sb, expall, psp))

            emit_oh()

            pending = []
            for i in range(LOOKAHEAD):
                emit_unit_dma(i)
            for i in range(NUNIT):
                # keep the scalar engine's DMA issues ahead of its exps —
                # the engine is in-order, so an exp emitted before a
                # dma_start would gate the stream on the compute pipeline.
                if i + LOOKAHEAD < NUNIT:
                    emit_unit_dma(i + LOOKAHEAD)
                emit_unit(i, i)
            while pending:
                emit_mult_mms(pending.pop(0))

            # ---- epilogue ----
            # denominator: rowsum over pall -> l per batch -> 1/l
            nc.scalar.activation(pscr_sb[:], pall_sb[:], AF.Copy,
                                 accum_out=rowsum_sb[:])
            nc.tensor.matmul(psum_l[:], rT_sb[:], rowsum_sb[:],
                             start=True, stop=True)
            nc.vector.reciprocal(linv_sb[:], psum_l[:])
            # unnormalized c_t: merge the two PE row-groups and transpose in
            # one matmul per 128-chunk of h (stationary ct4 chunk x s2).
            nc.scalar.copy(ct4_sb[:], psum_ct4[:])
            for hc in range(4):
                ptr = pptr.tile([128, BC], F32)
                nc.tensor.matmul(ptr[:], ct4_sb[:, hc * 128:(hc + 1) * 128],
                                 s2_sb[:], start=True, stop=True)
                nc.scalar.copy(catT_sb[:, hc * BC:(hc + 1) * BC], ptr[:])
            for ic in range(4):
                nc.tensor.matmul(psum_oc[:], catT_sb[:, ic * BC:(ic + 1) * BC],
                                 w_rT_sb[:, ic * H:(ic + 1) * H],
                                 start=(ic == 0), stop=(ic == 3))
            # out = tanh(oc * (1/l) + oh)
            nc.vector.scalar_tensor_tensor(o2_sb[:], psum_oc[:], linv_sb[:],
                                           oh_sb[:], ALU.mult, ALU.add)
            nc.scalar.activation(out_sb[:], o2_sb[:], AF.Tanh)
            nc.sync.dma_start(out[:], out_sb[:])

    nc.compile()
    return nc


def prep_in_maps(inputs):
    enc = np.asarray(inputs["encoder_hidden_states"]).astype(np.float32, copy=False)
    h_t = np.asarray(inputs["h_t"]).astype(np.float32, copy=False)
    mask = np.asarray(inputs["encoder_context_mask"]).astype(np.int32, copy=False)
    W_a = np.asarray(inputs["W_a"]).astype(np.float32, copy=False)
    W_r = np.asarray(inputs["W_r"]).astype(np.float32, copy=False)
    b_r = np.asarray(inputs["b_r"]).astype(np.float32, copy=False)

    ng = S // SS
    p_idx = np.arange(128)
    b_idx = np.arange(BC)
    # w_a packed [128, (c4, k512)] fp16
    w_a_p = np.ascontiguousarray(
        W_a.reshape(4, 128, H).transpose(1, 0, 2).reshape(128, 4 * H)
        .astype(np.float16))
    # w_rT packed [128, (c8, n512)] fp16
    w_rT_p = np.ascontiguousarray(
        W_r.T.reshape(8, 128, H).transpose(1, 0, 2).reshape(128, 8 * H)
        .astype(np.float16))
    r_mat = (p_idx[None, :] % BC == b_idx[:, None]).astype(np.float32)
    # the expall accum double-counts each partial denominator 8x (one per
    # broadcast dup column) — fold the 1/8 into the collapse matrix
    c32 = np.ascontiguousarray(np.concatenate(
        [r_mat.T / 8.0, np.full((128, 1), -C_SHIFT, np.float32)], axis=1))
    onehot = (p_idx[:, None] % BC == b_idx[None, :]).astype(np.float32)
    oneh_full = np.ascontiguousarray(
        np.tile(onehot, (1, GPP)).astype(ml_dtypes.bfloat16))
    s2_mat_np = np.zeros((128, BC), np.float32)
    for b in range(BC):
        s2_mat_np[b, b] = 1.0
        s2_mat_np[32 + b, b] = 1.0
    b_r_rep = np.broadcast_to(b_r, (BC, H)).astype(np.float32)
    p8 = np.ascontiguousarray(b_r_rep)
    rm16 = np.ascontiguousarray(r_mat.astype(np.float16))

    in_maps = []
    for c in range(NCORES):
        bs = slice(c * BC, (c + 1) * BC)
        mask_c = mask[bs, :]
        mask_p = np.ascontiguousarray(
            mask_c.reshape(BC, ng, SS).transpose(2, 0, 1).reshape(128, ng))
        h_c = h_t[bs]                                   # [BC, H]
        c16 = np.ascontiguousarray(
            h_c.T.reshape(4, 128, BC).transpose(1, 0, 2).reshape(128, 4 * BC)
            .astype(np.float16))
        in_maps.append({
            "enc": np.ascontiguousarray(
                enc[:, bs, :]
                .reshape(NPAIR, 2 * (S_TILE // SS), SS, BC, H)
                .transpose(0, 2, 3, 1, 4)
                .reshape(NPAIR, 128, PW)
                .astype(np.float16)),
            "w_a": w_a_p,
            "w_rT": w_rT_p,
            "c32": c32,
            "c16": c16,
            "p8": p8,
            "rm16": rm16,
            "s2m": s2_mat_np.astype(ml_dtypes.bfloat16),
            "oneh": oneh_full,
            "mask_p": mask_p,
        })
    return in_maps


_CACHE = {}


def _reset_device():
    # Best-effort recovery of a wedged NeuronCore left by a previous process.
    try:
        import ctypes
        lib = ctypes.CDLL("/opt/axon/libaxon_pjrt.so")
        lib.axon_reset.restype = ctypes.c_int64
        import jax
        jax.devices()
        lib.axon_reset()
    except Exception:
        pass


def run(inputs, trace=False, **kw):
    mask = np.asarray(inputs["encoder_context_mask"])
    with_logm = not bool((mask == 1).all())
    key = ("nc", with_logm)
    if key not in _CACHE:
        _CACHE[key] = build_program(with_logm=with_logm)
    nc = _CACHE[key]
    in_maps = prep_in_maps(inputs)
    if not with_logm:
        for m in in_maps:
            m.pop("mask_p", None)

    def _once():
        try:
            return run_bass_kernel_spmd(nc, in_maps, list(range(NCORES)),
                                        trace=trace, **kw)
        except Exception:
            _reset_device()
            return run_bass_kernel_spmd(nc, in_maps, list(range(NCORES)),
                                        trace=trace, **kw)

    # Rare timing-dependent glitch under heavy device throttling can corrupt
    # a run (NaN / out-of-range tanh output). Detect and retry.
    for attempt in range(3):
        res = _once()
        full = np.concatenate([np.asarray(res.results[c]["out"])
                               for c in range(NCORES)], axis=0).astype(np.float32)
        if np.isfinite(full).all() and np.abs(full).max() <= 1.0 + 1e-3:
            break
    return full, res


def kernel(**inputs):
    return run(inputs)[0]
